# revision 18
# baseline (speedup 1.0000x reference)
"""MultiHeadSelfAttention Trainium2 kernel (8 NeuronCores, SPMD).

Problem: x[2,2048,1024], H=16 heads, hd=64.  out = softmax(QK^T/8)V + x.

Sharding (tensor-parallel over heads x data-parallel over batch):
  core c (0..7): batch b = c//4, head group g = c%4 -> heads [4g, 4g+4),
  i.e. output columns [256g, 256g+256) of batch b.  No collectives.

Design (vs 210us baseline):
  - bk dropped: the K-bias score term Q_q.bk is constant over k, softmax
    is shift-invariant.  Exact.
  - bv folded into the residual input host-side (xres = x + bv): V-bias
    passes through the softmax-weighted average untouched.  Exact.
  - all matmul inputs except Q/K are fp8e4m3 (weights pre-scaled x16 so
    U(-1/32,1/32) values leave the subnormal range; scores come out
    256x, compensated in the exp scale).
  - projections use fp8 DoubleRow (2 D-tiles per matmul, 2 MACs/cell).
  - scores: the two heads of a pair run CONCURRENTLY as two K=64
    row-tiled bf16 matmuls (tile_position (0,0)/(64,0)).
  - exp split across two engines: ~half the kt tiles on ACT (exact exp,
    fp8 out), rest on DVE via Schraudolph bit-trick exp:
    i8 = rint(A*s + 55.5) bitcast fp8e4m3 ~= exp(s/2048) (+-3-8% per
    weight; softmax-weight errors mostly cancel in the weighted mean).
  - AV uses fp8 DoubleRow (2 k-tiles per matmul); V carries a 16.0
    column so 16*sum(exp) falls out of the same matmul (numerator is
    16V so the 16s cancel in the normalize).
  - normalize: 4 transposes per (head,qb) land in ONE psum bank
    [128,4,66]; one batched DVE reciprocal of the 4 sum columns; scale
    muls on ACT (per-partition scale AP); residual add on DVE.
  - software pipelined: AV+normalize of q-block qb-1 are emitted after
    the scores+exps of qb so the strict-FIFO ACT/DVE queues never
    head-of-line block on the normalize chain.
"""

import ml_dtypes
import numpy as np

B, S, D, H = 2, 2048, 1024, 16
HD = 64
NCORES = 8
GH = 4            # heads per core
GD = GH * HD      # 256 output columns per core
P = 128
DT = D // P       # 8 D-tiles (contraction)
KT = S // P       # 16 k-tiles
QB = 512          # query block
NQB = S // QB     # 4
NQT = S // P      # 16 query tiles of 128
VW = 80           # per-head V slot width (fp8, 16B-aligned for DoubleRow)

WSCALE = 16.0     # host pre-scale on Wq/Wk/Wv + bq (fp8 subnormal dodge)
SSCALE = 0.125 / (WSCALE * WSCALE)   # exp arg = score * SSCALE
# Schraudolph fp8e4m3 exp: exp(s*SSCALE) ~= bitcast_f8(i8(A*s + B))
SCH_A8 = 8.0 * SSCALE / float(np.log(2.0))
SCH_B8 = 7.0 * 8.0 - 0.5

_CACHE = {}
TRACE = False
LAST_RESULTS = None


def _build_nc():
    import concourse.bass as bass
    import concourse.mybir as mybir
    import concourse.tile as tile
    from concourse import bacc
    from concourse.masks import make_identity

    f32 = mybir.dt.float32
    bf16 = mybir.dt.bfloat16
    f8 = mybir.dt.float8e4
    i8 = mybir.dt.int8
    EXP = mybir.ActivationFunctionType.Exp
    MULT = mybir.AluOpType.mult
    ADD = mybir.AluOpType.add
    DR = mybir.MatmulPerfMode.DoubleRow

    nc = bacc.Bacc("TRN2")

    # wq|wk|wv|x^T packed into one fp8 DRAM tensor (single-DMA dep chains).
    xw_d = nc.dram_tensor("xw", [D, 3 * GD + S], f8, kind="ExternalInput")
    bq_d = nc.dram_tensor("bq", [GD], f32, kind="ExternalInput")
    xres_d = nc.dram_tensor("xres", [S, GD], f32, kind="ExternalInput")
    out_d = nc.dram_tensor("out", [S, GD], f32, kind="ExternalOutput")

    with tile.TileContext(nc) as tc:
        with (
            tc.tile_pool(name="persist", bufs=1) as persist,
            tc.tile_pool(name="exps_pool", bufs=3) as exps_pool,
            tc.tile_pool(name="work", bufs=3) as work,
            tc.tile_pool(name="psum", bufs=1, space="PSUM") as psum,
        ):
            # ---- constants / weights ----
            identity = persist.tile([P, P], f32, tag="identity")
            make_identity(nc, identity)

            bq_sb = persist.tile([P, 2], f32, tag="bq_sb")
            nc.sync.dma_start(bq_sb, bq_d.rearrange("(m p) -> p m", p=P))

            xw_sb = persist.tile([P, DT, 3 * GD + S], f8, tag="xw_sb")
            xw_r = xw_d.rearrange("(dt p) s -> p dt s", p=P)
            W0 = 3 * GD
            bounds = [0, W0 + QB, W0 + 2 * QB, W0 + 3 * QB, W0 + S]
            for c in range(4):
                nc.sync.dma_start(
                    xw_sb[:, :, bounds[c]:bounds[c + 1]],
                    xw_r[:, :, bounds[c]:bounds[c + 1]],
                )
            wq_sb = xw_sb[:, :, 0:GD]
            wk_sb = xw_sb[:, :, GD:2 * GD]
            wv_sb = xw_sb[:, :, 2 * GD:3 * GD]
            xT_sb = xw_sb[:, :, 3 * GD:]

            # Pre-observe the bq DMA so downstream consumers don't carry a
            # DMA wait alongside a PE wait.
            sink = persist.tile([P, 2], f32, tag="sink")
            nc.vector.tensor_copy(sink[:, 0:1], bq_sb[:, 0:1])

            # ---- persistent activations ----
            # qT/kT: heads 2m/2m+1 packed on partition halves of M-tile m.
            qT_sb = persist.tile([P, 2, S], bf16, tag="qT_sb")
            kT_sb = persist.tile([P, 2, S], bf16, tag="kT_sb")
            v_sb = persist.tile([P, KT, GH * VW], f8, tag="v_sb")
            out_sb = persist.tile([P, NQT, GD], f32, tag="out_sb")

            # 16.0 columns for the 16*sum(exp) trick
            v_w = v_sb.rearrange("p t (h c) -> p t h c", c=VW)
            nc.vector.memset(v_w[:, :, :, HD:HD + 1], 16.0)

            def emit_proj_chunk(m, q2, is_q):
                """One 1024-token Q or K projection chunk for M-tile m.

                fp8 DoubleRow: contract 2 D-tiles per matmul."""
                w_sb, dst = (wq_sb, qT_sb) if is_q else (wk_sb, kT_sb)
                ps = psum.tile([P, 1024], f32, tag="ps_big", bufs=3,
                               name="ps_proj")
                for half in range(2):
                    tok = (q2 * 2 + half) * QB
                    for dp in range(DT // 2):
                        nc.tensor.matmul(
                            ps[:, half * QB:(half + 1) * QB],
                            lhsT=w_sb[:, 2 * dp:2 * dp + 2,
                                      m * P:(m + 1) * P],
                            rhs=xT_sb[:, 2 * dp:2 * dp + 2, tok:tok + QB],
                            start=(dp == 0), stop=(dp == DT // 2 - 1),
                            perf_mode=DR,
                        )
                sl = slice(q2 * 1024, (q2 + 1) * 1024)
                if is_q:
                    # evac on ACT with fused per-partition bias
                    nc.scalar.add(dst[:, m, sl], ps, bq_sb[:, m:m + 1])
                else:
                    nc.vector.tensor_copy(dst[:, m, sl], ps)

            def emit_qk(m):
                emit_proj_chunk(m, 0, False)
                emit_proj_chunk(m, 1, False)
                emit_proj_chunk(m, 0, True)
                emit_proj_chunk(m, 1, True)

            def emit_v(tiles):
                for tt in tiles:
                    ps = psum.tile([P, GD], f32, tag="ps_x", bufs=2,
                                   name="ps_v")
                    for dp in range(DT // 2):
                        nc.tensor.matmul(
                            ps,
                            lhsT=xT_sb[:, 2 * dp:2 * dp + 2,
                                       tt * P:(tt + 1) * P],
                            rhs=wv_sb[:, 2 * dp:2 * dp + 2, :],
                            start=(dp == 0), stop=(dp == DT // 2 - 1),
                            perf_mode=DR,
                        )
                    # fp8-quantizing strided evac (V slots are VW wide)
                    nc.scalar.copy(
                        v_w[:, tt, :, :HD],
                        ps.rearrange("p (h c) -> p h c", c=HD),
                    )

            def finalize_qt(qt):
                # residual add on the otherwise-idle GpSimd engine
                xr = work.tile([P, GD], f32, tag="xr", name="xr")
                nc.sync.dma_start(xr, xres_d[qt * P:(qt + 1) * P, :])
                nc.gpsimd.tensor_add(out_sb[:, qt, :], out_sb[:, qt, :], xr)
                nc.sync.dma_start(out_d[qt * P:(qt + 1) * P, :], out_sb[:, qt, :])

            def emit_pair(m, finalize=False, pending=None, fillers={}):
                """Attention for the head pair of M-tile m (heads 2m, 2m+1).

                Software-pipelined: av_norm(qb-1) is emitted after
                scores_exps(qb).  The last av_norm is RETURNED as a closure
                so the caller can defer it past the next pair's first
                scores (keeps ACT/DVE busy across the pair boundary).
                `pending` is such a closure from the previous pair."""
                def scores_exps(qb):
                    q0 = qb * QB
                    # exps[p, kt, head-in-pair, q]
                    exps = exps_pool.tile([P, KT, 2, QB], f8, tag="exps",
                                          name="exps")
                    exps_i = exps.bitcast(i8)
                    for kt in range(KT):
                        pss = psum.tile([P, 1024], f32, tag="ps_big", bufs=3,
                                        name="ps_s")
                        # two concurrent K=64 row-tiled matmuls: head 2m on
                        # array rows 0-63, head 2m+1 on rows 64-127
                        nc.tensor.matmul(
                            pss[:, 0:QB],
                            lhsT=kT_sb[:HD, m, kt * P:(kt + 1) * P],
                            rhs=qT_sb[:HD, m, q0:q0 + QB],
                            start=True, stop=True,
                        )
                        nc.tensor.matmul(
                            pss[:, QB:2 * QB],
                            lhsT=kT_sb[HD:, m, kt * P:(kt + 1) * P],
                            rhs=qT_sb[HD:, m, q0:q0 + QB],
                            start=True, stop=True,
                        )
                        if kt % 2 == 0 or kt == 15:
                            nc.scalar.activation(
                                out=exps[:, kt, :, :],
                                in_=pss, func=EXP, scale=SSCALE,
                            )
                        else:
                            nc.vector.tensor_scalar(
                                out=exps_i[:, kt, :, :],
                                in0=pss,
                                scalar1=SCH_A8, scalar2=SCH_B8,
                                op0=MULT, op1=ADD,
                            )
                    return exps

                def av_norm(qb, exps, on_act=False):
                    for hh in range(2):
                        head = 2 * m + hh
                        vcol = head * VW
                        pso = psum.tile([HD + 1, QB], f32, tag="ps_x",
                                        bufs=2, name="ps_o")
                        # fp8 DoubleRow AV: 2 k-tiles per matmul
                        for t2 in range(KT // 2):
                            nc.tensor.matmul(
                                pso,
                                lhsT=v_sb[:, 2 * t2:2 * t2 + 2,
                                          vcol:vcol + HD + 1],
                                rhs=exps[:, 2 * t2:2 * t2 + 2, hh, :],
                                start=(t2 == 0), stop=(t2 == KT // 2 - 1),
                                perf_mode=DR,
                            )
                        oT = work.tile([HD + 1, QB], f32, tag="oT", name="oT")
                        if on_act:
                            nc.scalar.copy(oT, pso)
                        else:
                            nc.vector.tensor_copy(oT, pso)
                        # 4 transposes into one bank; batched recip
                        pst = psum.tile([P, 4, 66], f32, tag="ps_x",
                                        bufs=2, name="ps_t")
                        for q4 in range(QB // P):
                            nc.tensor.transpose(
                                pst[:, q4, 0:HD + 1],
                                oT[:, q4 * P:(q4 + 1) * P],
                                identity[:HD + 1, :HD + 1],
                            )
                        r4 = work.tile([P, 4], f32, tag="r4", name="r4")
                        nc.vector.reciprocal(r4, pst[:, :, HD])
                        if on_act:
                            for q4 in range(QB // P):
                                qt = qb * (QB // P) + q4
                                nc.scalar.mul(
                                    out_sb[:, qt, head * HD:(head + 1) * HD],
                                    pst[:, q4, :HD], r4[:, q4:q4 + 1],
                                )
                        else:
                            # one broadcast tensor_tensor: out[q4,d] = pst*r4
                            nc.vector.tensor_tensor(
                                out_sb[:, qb * 4:(qb + 1) * 4,
                                       head * HD:(head + 1) * HD],
                                pst[:, :, :HD],
                                r4[:, :, None].broadcast_to([P, 4, HD]),
                                MULT,
                            )
                    if finalize:
                        for q4 in range(QB // P):
                            finalize_qt(qb * (QB // P) + q4)

                prev = None
                for qb in range(NQB):
                    cur = scores_exps(qb)
                    if qb == 0 and pending is not None:
                        pending()
                    for fill in fillers.get(qb, ()):
                        fill()
                    if prev is not None:
                        av_norm(qb - 1, prev, on_act=(finalize and qb == 3))
                    prev = cur
                last = prev
                return lambda: av_norm(NQB - 1, last, on_act=finalize)

            emit_qk(0)
            # V projection and the m=1 Q/K projections are interleaved into
            # pair 0's q-block loop as fillers (PE slack under the
            # exp-engine-bound attention phases).
            tail0 = emit_pair(0, fillers={
                0: [lambda: emit_v(range(0, 8))],
                1: [lambda: emit_v(range(8, KT))],
                2: [lambda: emit_proj_chunk(1, 0, False)],
                3: [lambda: emit_proj_chunk(1, 1, False),
                    lambda: emit_proj_chunk(1, 0, True)],
            })
            emit_proj_chunk(1, 1, True)
            tail1 = emit_pair(1, finalize=True, pending=tail0)
            tail1()

    nc.finalize()
    return nc


def _get_nc():
    if "nc" not in _CACHE:
        _CACHE["nc"] = _build_nc()
    return _CACHE["nc"]


def kernel(x, Wq, bq, Wk, bk, Wv, bv):
    global LAST_RESULTS
    from concourse.bass_utils import run_bass_kernel_spmd

    x = np.asarray(x, dtype=np.float32)
    Wq, Wk, Wv = (np.asarray(a, dtype=np.float32) for a in (Wq, Wk, Wv))
    bq, bv = (np.asarray(a, dtype=np.float32) for a in (bq, bv))

    f8 = ml_dtypes.float8_e4m3
    xTs = [x[b].T for b in range(B)]
    in_maps = []
    for c in range(NCORES):
        b, g = c // 4, c % 4
        cols = slice(GD * g, GD * (g + 1))
        xw = np.concatenate(
            [WSCALE * Wq[:, cols], WSCALE * Wk[:, cols],
             WSCALE * Wv[:, cols], xTs[b]], axis=1)
        xw = np.clip(xw, -240.0, 240.0).astype(f8)
        in_maps.append({
            "xw": xw,
            "bq": np.ascontiguousarray(WSCALE * bq[cols]),
            "xres": np.ascontiguousarray(x[b][:, cols] + bv[cols]),
        })

    nc = _get_nc()
    res = run_bass_kernel_spmd(
        nc, in_maps, core_ids=list(range(NCORES)), trace=TRACE,
    )
    LAST_RESULTS = res

    full = np.empty((B, S, D), dtype=np.float32)
    for c in range(NCORES):
        b, g = c // 4, c % 4
        full[b, :, GD * g:GD * (g + 1)] = res.results[c]["out"]
    return full
